# revision 7
# baseline (speedup 1.0000x reference)
"""DiffNet Trainium2 kernel: 5 iterations of a 4-layer CNN (BN training-mode,
stride-1 maxpools) gating a neighbor-diffusion update on x[16,1,256,256].

Sharding: data-parallel, 2 images per NeuronCore across 8 cores. BN batch
stats are synchronized with one tiny AllGather per BN layer (15 total).

Layout: activations live in SBUF as [128 partitions = 32 channels x 4 groups,
row-major padded slab]. Group g = half*2 + image (h-major so each half is a
contiguous 64-partition range). Convs run as 9 accumulating matmuls (one per
3x3 tap) with shifted rhs access patterns against a block-diagonal stationary.
Pools run on raw conv outputs (valid since the BN affine has positive scale),
fused with the BN normalize+relu into the eviction to the next layer's slab.
"""
import numpy as np
import ml_dtypes

BF16NP = ml_dtypes.bfloat16

# ---------------------------------------------------------------------------
# walrus-workaround patches (this build accepts only ONE sync wait per
# instruction; Tile attaches several). Split extras onto injected NoOps.
# ---------------------------------------------------------------------------
_PATCHED = False


def _install_patches():
    global _PATCHED
    if _PATCHED:
        return
    import concourse.tile as tile
    import concourse.mybir as mybir
    import bass_rust
    from concourse.vector_clock import ScopedClock
    from concourse import tile_utils

    tile_utils.max_sbuf_usage = 207 * 1024  # default 192K leaves room unused

    _orig_lower = tile.TileContext._lower_ordered_insts
    counter = [0]

    def _patched_lower(self, ordered):
        for bb_name, insts in list(ordered.items()):
            new = []
            changed = False
            for inst in insts:
                try:
                    si = inst.sync_info
                except Exception:
                    si = None
                waits = list(si.on_wait) if si is not None else []
                if len(waits) > 1:
                    for w in waits[:-1]:
                        nop = mybir.InstNoOp(name=f"wsplit_{counter[0]}")
                        counter[0] += 1
                        nop.engine = inst.engine
                        nop.sync_info = bass_rust.SyncInfo(on_wait=[w], on_update=[])
                        new.append(nop)
                    inst.sync_info = bass_rust.SyncInfo(
                        on_wait=[waits[-1]], on_update=list(si.on_update)
                    )
                    changed = True
                new.append(inst)
            if changed:
                ordered[bb_name] = new
        return _orig_lower(self, ordered)

    def _patched_drain(self, tick_clock, wait_clock):
        collector = self.nc.sync.drain()
        wait_clock.add_sem_waits(
            collector.ins, ScopedClock({None: tick_clock.global_clock})
        )
        waits = list(collector.ins.sync_info.on_wait)
        if len(waits) > 1:
            collector.ins.sync_info = bass_rust.SyncInfo(
                on_wait=[waits[0]], on_update=[]
            )
            for w in waits[1:]:
                d = self.nc.sync.drain()
                d.ins.sync_info = bass_rust.SyncInfo(on_wait=[w], on_update=[])
        self.nc.all_engine_barrier()
        popped = self.nc._tile_sem_poison_stack.pop()
        assert popped is self._sem_poison
        self.nc.clear_and_free_semaphores(list(self.sems.allocated().values()))
        self.nc.all_engine_barrier()

    tile.TileContext._lower_ordered_insts = _patched_lower
    tile.TileContext._drain_and_barrier = _patched_drain
    _PATCHED = True


# ---------------------------------------------------------------------------
# problem constants
# ---------------------------------------------------------------------------
L = 5
CNN_C = 32
META_C = 5
B = 16
H = 256
EPS = 1e-5
N_CORES = 8
NEG = -1e30

HL = [256, 257, 258, 259]          # conv input/output H (=W) for layers 1..4
AL = [128, 129, 129, 129]          # rows in half 0
BL = [HL[i] - AL[i] for i in range(4)]   # rows in half 1 (128,128,129,130)
RL = [max(AL[i], BL[i]) for i in range(4)]
WPL = [HL[i] + 2 for i in range(4)]
SLAB = (RL[3] + 2) * WPL[3]        # 132*261 = 34452 flat elems


def _build_kernel(dt_val):
    import concourse.bass as bass
    import concourse.tile as tile
    from concourse import mybir

    FP32 = mybir.dt.float32
    BF16 = mybir.dt.bfloat16
    ALU = mybir.AluOpType
    ACTF = mybir.ActivationFunctionType
    AXIS = mybir.AxisListType

    nc = bass.Bass("TRN2", target_bir_lowering=False, debug=False,
                   num_devices=N_CORES)

    x_in = nc.dram_tensor("x", [2, 256, 256], FP32, kind="ExternalInput").ap()
    wc1 = nc.dram_tensor("wc1", [4, 45 * 128], BF16, kind="ExternalInput").ap()
    wc2 = nc.dram_tensor("wc2", [128, 45 * 128], BF16, kind="ExternalInput").ap()
    wc3 = nc.dram_tensor("wc3", [128, 45 * 128], BF16, kind="ExternalInput").ap()
    wc4 = nc.dram_tensor("wc4", [128, 45 * 20], BF16, kind="ExternalInput").ap()
    sel_in = nc.dram_tensor("sel", [128, 32], FP32, kind="ExternalInput").ap()
    g_in = nc.dram_tensor("gv", [32, 15], FP32, kind="ExternalInput").ap()
    be_in = nc.dram_tensor("bev", [32, 15], FP32, kind="ExternalInput").ap()
    b4_in = nc.dram_tensor("b4v", [20, 5], FP32, kind="ExternalInput").ap()
    y_out = nc.dram_tensor("y", [2, 256, 256], FP32, kind="ExternalOutput").ap()

    with tile.TileContext(nc) as tc:
        with (
            tc.tile_pool(name="big", bufs=1) as bigp,
            tc.tile_pool(name="wp", bufs=1) as wp,
            tc.tile_pool(name="xp", bufs=2) as xp,
            tc.tile_pool(name="mp", bufs=1) as mp,
            tc.tile_pool(name="sp", bufs=8) as sp,
            tc.tile_pool(name="cv", bufs=3, space="PSUM") as cvp,
            tc.tile_pool(name="upp", bufs=2, space="PSUM") as upp,
            tc.tile_pool(name="slp", bufs=1, space="PSUM") as slp,
            tc.tile_pool(name="dr", bufs=4, space="DRAM") as drp,
        ):
            S1 = bigp.tile([128, SLAB], BF16, tag="S1")
            S2 = bigp.tile([128, SLAB], BF16, tag="S2")
            w1t = wp.tile([4, 45 * 128], BF16)
            w2t = wp.tile([128, 45 * 128], BF16)
            w3t = wp.tile([128, 45 * 128], BF16)
            w4t = wp.tile([128, 45 * 20], BF16)
            selt = wp.tile([128, 32], FP32)
            gt = wp.tile([32, 15], FP32)
            bet = wp.tile([32, 15], FP32)
            b4t = wp.tile([20, 5], FP32)
            epst = wp.tile([32, 1], FP32)
            nc.vector.memset(epst[:], EPS)
            meta = mp.tile([128, 5, 4, 256], BF16)

            nc.sync.dma_start(out=w1t[:], in_=wc1[:])
            nc.sync.dma_start(out=w2t[:], in_=wc2[:])
            nc.sync.dma_start(out=w3t[:], in_=wc3[:])
            nc.sync.dma_start(out=w4t[:], in_=wc4[:])
            nc.sync.dma_start(out=selt[:], in_=sel_in[:])
            nc.sync.dma_start(out=gt[:], in_=g_in[:])
            nc.sync.dma_start(out=bet[:], in_=be_in[:])
            nc.sync.dma_start(out=b4t[:], in_=b4_in[:])

            # ---------------- xgrid setup ----------------
            def refresh_xg_halos(xg):
                xv = xg
                # zero all halo slots first (aligned full-range memsets);
                # the DMAs below overwrite where a real neighbor row exists.
                nc.vector.memset(xv[:, :, 0:1], 0.0)
                nc.vector.memset(xv[:, :, 257:258], 0.0)
                nc.vector.memset(xv[:, 0:1, :], 0.0)
                nc.vector.memset(xv[:, 5:6, :], 0.0)
                for g in range(4):
                    p0 = 32 * g
                    # top halo within group: part q row0 <- part q-1 row4
                    nc.sync.dma_start(out=xv[p0 + 1:p0 + 32, 0:1, :],
                                      in_=xv[p0:p0 + 31, 4:5, :])
                    # bottom halo within group: part q row5 <- part q+1 row1
                    nc.sync.dma_start(out=xv[p0:p0 + 31, 5:6, :],
                                      in_=xv[p0 + 1:p0 + 32, 1:2, :])
                for g in (2, 3):  # h1 first block: top halo <- h0 last row
                    src = (g - 2) * 32 + 31
                    nc.sync.dma_start(out=xv[g * 32:g * 32 + 1, 0:1, :],
                                      in_=xv[src:src + 1, 4:5, :])
                for g in (0, 1):  # h0 last block: bottom halo <- h1 first row
                    dst = g * 32 + 31
                    src = (g + 2) * 32
                    nc.sync.dma_start(out=xv[dst:dst + 1, 5:6, :],
                                      in_=xv[src:src + 1, 1:2, :])

            def build_slab1(xg):
                # write x into S1 partitions 0..3 as the layer-1 input slab
                wp1 = WPL[0]
                s1v = S1[:, 0:130 * wp1].rearrange("p (r c) -> p r c", c=wp1)
                # zero halo rows first (aligned memset), then fill real seams
                nc.vector.memset(s1v[0:4, 0:1, :], 0.0)
                nc.vector.memset(s1v[0:4, 129:130, :], 0.0)
                for g in range(4):
                    nc.gpsimd.dma_start(out=s1v[g:g + 1, 1:129, :],
                                        in_=xg[32 * g:32 * g + 32, 1:5, :])
                nc.gpsimd.dma_start(out=s1v[2:3, 0:1, :], in_=xg[31:32, 4:5, :])
                nc.gpsimd.dma_start(out=s1v[3:4, 0:1, :], in_=xg[63:64, 4:5, :])
                nc.gpsimd.dma_start(out=s1v[0:1, 129:130, :], in_=xg[64:65, 1:2, :])
                nc.gpsimd.dma_start(out=s1v[1:2, 129:130, :], in_=xg[96:97, 1:2, :])

            xg = xp.tile([128, 6, 258], FP32, tag="xg")
            nc.vector.memset(xg[:], 0.0)
            for g in range(4):
                hh, img = g >> 1, g & 1
                nc.sync.dma_start(
                    out=xg[32 * g:32 * g + 32, 1:5, 1:257],
                    in_=x_in[img, hh * 128:(hh + 1) * 128, :])
            refresh_xg_halos(xg)
            build_slab1(xg)

            # ---------------- per-layer emitters ----------------
            def conv_layer(li, it):
                """li: 0..3 (layers 1..4). Reads S1, writes raw into S2."""
                wpad = WPL[li]
                rr = RL[li]
                base = wpad + 1
                total = rr * wpad - 2
                idx0 = it * 9
                if li == 0:
                    wt, kp, mp_ = w1t, 4, 128
                elif li == 1:
                    wt, kp, mp_ = w2t, 128, 128
                elif li == 2:
                    wt, kp, mp_ = w3t, 128, 128
                else:
                    wt, kp, mp_ = w4t, 128, 20
                msz = 128 if li < 3 else 20
                off = 0
                while off < total:
                    n = min(512, total - off)
                    pt = cvp.tile([128, 512], mybir.dt.float32, tag="cv")
                    for t in range(9):
                        dy, dx = t // 3 - 1, t % 3 - 1
                        ro = base + off + dy * wpad + dx
                        nc.tensor.matmul(
                            pt[0:msz, 0:n],
                            wt[0:kp, (idx0 + t) * mp_:(idx0 + t) * mp_ + msz],
                            S1[0:kp, ro:ro + n],
                            start=(t == 0), stop=(t == 8),
                        )
                    if li < 3:
                        nc.scalar.activation(
                            S2[:, base + off:base + off + n], pt[:, 0:n],
                            mybir.ActivationFunctionType.Copy)
                    else:
                        nc.scalar.activation(
                            S2[0:20, base + off:base + off + n], pt[0:20, 0:n],
                            mybir.ActivationFunctionType.Relu,
                            bias=b4t[:, it:it + 1], scale=1.0)
                    off += n

            def stats_pool_norm(li, it):
                """Stats + AllGather + affine; pool raw; normalized -> S1."""
                wpad, rr = WPL[li], RL[li]
                a, b = AL[li], BL[li]
                w = HL[li]
                a2, b2 = AL[li + 1], BL[li + 1]
                r2, wp2 = RL[li + 1], WPL[li + 1]
                w2 = HL[li + 1]
                nfull = 16 * w * w
                s2v = S2[:, 0:(rr + 2) * wpad].rearrange("p (r c) -> p r c", c=wpad)

                # per-partition sums over real pixels (fp32 accumulate)
                sums = sp.tile([128, 2], mybir.dt.float32, tag="sums")
                nc.vector.tensor_reduce(sums[0:64, 0:1], s2v[0:64, 1:1 + a, 1:1 + w],
                                        AXIS.XY, ALU.add)
                nc.vector.tensor_reduce(sums[64:128, 0:1], s2v[64:128, 1:1 + b, 1:1 + w],
                                        AXIS.XY, ALU.add)

                # pool prep on raw: -inf halos
                nc.vector.memset(s2v[:, 1:rr + 1, 0:1], NEG)
                nc.vector.memset(s2v[:, 1:rr + 1, w + 1:w + 2], NEG)
                nc.vector.memset(s2v[0:64, 0:1, :], NEG)          # h0 top pad
                nc.sync.dma_start(out=s2v[64:128, 0:1, :],        # h1 top <- h0 last
                                  in_=s2v[0:64, a:a + 1, :])
                nc.sync.dma_start(out=s2v[0:64, a + 1:a + 2, :],  # h0 bottom <- h1 first
                                  in_=s2v[64:128, 1:2, :])
                if b + 1 <= rr + 1:                               # h1 bottom pad(s)
                    nc.vector.memset(s2v[64:128, b + 1:rr + 2, :], NEG)

                # rowmax into S1 (input slab is dead after the convs)
                s1p = S1[:, 0:(rr + 1) * wpad].rearrange("p (r c) -> p r c", c=wpad)
                nc.vector.tensor_max(s1p[:, 0:rr + 1, :], s2v[:, 0:rr + 1, :],
                                     s2v[:, 1:rr + 2, :])

                # sum of squares: in-place square of raw (raw is dead after rowmax)
                nc.vector.scalar_tensor_tensor(
                    out=s2v[0:64, 1:1 + a, 1:1 + w],
                    in0=s2v[0:64, 1:1 + a, 1:1 + w], scalar=1.0,
                    in1=s2v[0:64, 1:1 + a, 1:1 + w],
                    op0=ALU.mult, op1=ALU.mult, accum_out=sums[0:64, 1:2])
                nc.vector.scalar_tensor_tensor(
                    out=s2v[64:128, 1:1 + b, 1:1 + w],
                    in0=s2v[64:128, 1:1 + b, 1:1 + w], scalar=1.0,
                    in1=s2v[64:128, 1:1 + b, 1:1 + w],
                    op0=ALU.mult, op1=ALU.mult, accum_out=sums[64:128, 1:2])

                # cross-group reduce via selector matmul, then AllGather
                chan = slp.tile([32, 2], mybir.dt.float32, tag="sl")
                nc.tensor.matmul(chan[:, :], selt[:, :], sums[:, :],
                                 start=True, stop=True)
                csb = sp.tile([32, 2], mybir.dt.float32, tag="csb")
                nc.vector.tensor_copy(csb[:], chan[:, :])
                ccin = drp.tile([64], mybir.dt.float32, tag="ccin")
                ccout = drp.tile([512], mybir.dt.float32, tag="ccout")
                nc.gpsimd.dma_start(out=ccin[:], in_=csb[:])
                nc.gpsimd.collective_compute(
                    "AllGather", ALU.bypass,
                    replica_groups=[list(range(N_CORES))],
                    ins=[ccin[:].opt()], outs=[ccout[:].opt()])
                gath = sp.tile([32, 2, 8], mybir.dt.float32, tag="gath")
                oap = ccout[:]
                src = bass.AP(tensor=oap.tensor, offset=oap.offset,
                              ap=[[2, 32], [1, 2], [64, 8]])
                nc.sync.dma_start(out=gath[:], in_=src)
                tot = sp.tile([32, 2], mybir.dt.float32, tag="tot")
                nc.vector.tensor_reduce(tot[:], gath[:], AXIS.X, ALU.add)

                # affine: scale = g / sqrt(var+eps); bias = be - mean*scale
                af = sp.tile([32, 6], mybir.dt.float32, tag="af")
                col = it * 3 + li
                nc.vector.tensor_scalar_mul(af[:, 0:1], tot[:, 0:1], 1.0 / nfull)
                nc.vector.tensor_scalar_mul(af[:, 1:2], tot[:, 1:2], 1.0 / nfull)
                nc.vector.tensor_mul(af[:, 2:3], af[:, 0:1], af[:, 0:1])
                nc.vector.tensor_sub(af[:, 1:2], af[:, 1:2], af[:, 2:3])  # var
                nc.scalar.activation(af[:, 2:3], af[:, 1:2],
                                     ACTF.Sqrt, bias=epst[:], scale=1.0)
                nc.vector.reciprocal(af[:, 3:4], af[:, 2:3])
                aff = sp.tile([32, 2], mybir.dt.float32, tag="aff")
                nc.vector.tensor_mul(aff[:, 0:1], af[:, 3:4], gt[:, col:col + 1])
                nc.vector.tensor_mul(af[:, 4:5], af[:, 0:1], aff[:, 0:1])
                nc.vector.tensor_sub(aff[:, 1:2], bet[:, col:col + 1], af[:, 4:5])
                aff128 = sp.tile([128, 2], mybir.dt.float32, tag="aff128")
                for g in range(4):
                    nc.sync.dma_start(out=aff128[32 * g:32 * g + 32, :], in_=aff[:])

                # colmax: prow (in S1) -> pooled (in S2, next-layer geometry)
                s2n = S2[:, 0:(r2 + 2) * wp2].rearrange("p (r c) -> p r c", c=wp2)
                nc.vector.tensor_max(s2n[0:64, 1:1 + a2, 1:1 + w2],
                                     s1p[0:64, 0:a2, 0:w2],
                                     s1p[0:64, 0:a2, 1:1 + w2])
                d = a2 - a
                nc.vector.tensor_max(s2n[64:128, 1:1 + b2, 1:1 + w2],
                                     s1p[64:128, d:d + b2, 0:w2],
                                     s1p[64:128, d:d + b2, 1:1 + w2])

                # normalize+relu pooled -> S1 (next input slab)
                s1n = S1[:, 0:(r2 + 2) * wp2].rearrange("p (r c) -> p r c", c=wp2)
                nc.scalar.activation(s1n[0:64, 1:1 + a2, 1:1 + w2],
                                     s2n[0:64, 1:1 + a2, 1:1 + w2],
                                     ACTF.Relu, bias=aff128[0:64, 1:2],
                                     scale=aff128[0:64, 0:1])
                nc.scalar.activation(s1n[64:128, 1:1 + b2, 1:1 + w2],
                                     s2n[64:128, 1:1 + b2, 1:1 + w2],
                                     ACTF.Relu, bias=aff128[64:128, 1:2],
                                     scale=aff128[64:128, 0:1])

                # next-slab halos (zeros for conv pad; cross copies at half seam)
                nc.vector.memset(s1n[:, 1:r2 + 1, 0:1], 0.0)
                nc.vector.memset(s1n[:, 1:r2 + 1, w2 + 1:w2 + 2], 0.0)
                nc.vector.memset(s1n[0:64, 0:1, :], 0.0)
                if b2 + 1 <= r2 + 1:
                    nc.vector.memset(s1n[64:128, b2 + 1:r2 + 2, :], 0.0)
                nc.sync.dma_start(out=s1n[64:128, 0:1, :],
                                  in_=s1n[0:64, a2:a2 + 1, :])
                nc.sync.dma_start(out=s1n[0:64, a2 + 1:a2 + 2, :],
                                  in_=s1n[64:128, 1:2, :])

            def repack_meta(it):
                wp4 = WPL[3]
                for g in range(4):
                    hh = g >> 1
                    row0 = (2 if hh == 0 else 1)
                    for k in range(5):
                        p = 5 * g + k
                        src = S2[p:p + 1, row0 * wp4:(row0 + 128) * wp4] \
                            .rearrange("p (j rr c) -> p j rr c", rr=4, c=wp4)
                        nc.sync.dma_start(
                            out=meta[32 * g:32 * g + 32, k, :, :],
                            in_=src[:, :, :, 2:258])

            def update_x(it, xg_cur):
                xn = xp.tile([128, 6, 258], mybir.dt.float32, tag="xg")
                xc = xg_cur[:, 1:5, 1:257]
                up = xg_cur[:, 0:4, 1:257]
                dn = xg_cur[:, 2:6, 1:257]
                lf = xg_cur[:, 1:5, 2:258]
                rt = xg_cur[:, 1:5, 0:256]
                acc = upp.tile([128, 4, 256], mybir.dt.float32, tag="up")
                tmp = xp.tile([128, 4, 256], mybir.dt.float32, tag="ut")
                STT = nc.vector.scalar_tensor_tensor
                STT(out=acc[:], in0=meta[:, 1, :, :], scalar=dt_val, in1=up,
                    op0=ALU.mult, op1=ALU.mult)
                STT(out=tmp[:], in0=meta[:, 2, :, :], scalar=dt_val, in1=dn,
                    op0=ALU.mult, op1=ALU.mult)
                nc.vector.tensor_add(acc[:], acc[:], tmp[:])
                STT(out=tmp[:], in0=meta[:, 3, :, :], scalar=dt_val, in1=lf,
                    op0=ALU.mult, op1=ALU.mult)
                nc.vector.tensor_add(acc[:], acc[:], tmp[:])
                STT(out=tmp[:], in0=meta[:, 4, :, :], scalar=dt_val, in1=rt,
                    op0=ALU.mult, op1=ALU.mult)
                nc.vector.tensor_add(acc[:], acc[:], tmp[:])
                STT(out=tmp[:], in0=meta[:, 0, :, :], scalar=dt_val, in1=xc,
                    op0=ALU.mult, op1=ALU.mult)
                nc.vector.tensor_add(tmp[:], tmp[:], xc)
                nc.vector.tensor_sub(xn[:, 1:5, 1:257], tmp[:], acc[:])
                if it == L - 1:
                    nc.vector.tensor_scalar_max(xn[:, 1:5, 1:257],
                                                xn[:, 1:5, 1:257], 0.0)
                return xn

            # ---------------- main loop ----------------
            for it in range(L):
                for li in range(3):
                    conv_layer(li, it)
                    stats_pool_norm(li, it)
                conv_layer(3, it)
                repack_meta(it)
                xg_new = update_x(it, xg)
                if it == L - 1:
                    for g in range(4):
                        hh, img = g >> 1, g & 1
                        nc.sync.dma_start(
                            out=y_out[img, hh * 128:(hh + 1) * 128, :],
                            in_=xg_new[32 * g:32 * g + 32, 1:5, 1:257])
                else:
                    refresh_xg_halos(xg_new)
                    build_slab1(xg_new)
                xg = xg_new

    return nc


def _prep_inputs(w1, w2, w3, w4, b4):
    """Build device-side stationary layouts (bf16) from torch-layout weights."""
    wc1 = np.zeros((4, 45, 128), np.float32)
    wc2 = np.zeros((128, 45, 128), np.float32)
    wc3 = np.zeros((128, 45, 128), np.float32)
    wc4 = np.zeros((128, 45, 20), np.float32)
    for i in range(L):
        for t in range(9):
            dy, dx = t // 3, t % 3
            s = i * 9 + t
            for g in range(4):
                wc1[g, s, 32 * g:32 * g + 32] = w1[i, :, 0, dy, dx]
                wc2[32 * g:32 * g + 32, s, 32 * g:32 * g + 32] = \
                    w2[i, :, :, dy, dx].T
                wc3[32 * g:32 * g + 32, s, 32 * g:32 * g + 32] = \
                    w3[i, :, :, dy, dx].T
                wc4[32 * g:32 * g + 32, s, 5 * g:5 * g + 5] = \
                    w4[i, :, :, dy, dx].T
    sel = np.zeros((128, 32), np.float32)
    for p in range(128):
        sel[p, p % 32] = 1.0
    b4v = np.zeros((20, 5), np.float32)
    for i in range(L):
        for g in range(4):
            b4v[5 * g:5 * g + 5, i] = b4[i]
    return (wc1.reshape(4, -1).astype(BF16NP),
            wc2.reshape(128, -1).astype(BF16NP),
            wc3.reshape(128, -1).astype(BF16NP),
            wc4.reshape(128, -1).astype(BF16NP),
            sel, b4v)


def kernel(x, w1, b1, g1, be1, w2, b2, g2, be2, w3, b3, g3, be3, w4, b4, dt):
    _install_patches()
    from concourse.bass_utils import run_bass_kernel_spmd

    x = np.asarray(x, np.float32)
    wc1, wc2, wc3, wc4, sel, b4v = _prep_inputs(
        np.asarray(w1, np.float32), np.asarray(w2, np.float32),
        np.asarray(w3, np.float32), np.asarray(w4, np.float32),
        np.asarray(b4, np.float32))
    # conv biases b1..b3 cancel inside train-mode BN; g/be enter the affine.
    gv = np.zeros((32, 15), np.float32)
    bev = np.zeros((32, 15), np.float32)
    for i in range(L):
        for li, (gg, bb) in enumerate(((g1, be1), (g2, be2), (g3, be3))):
            gv[:, i * 3 + li] = np.asarray(gg, np.float32)[i]
            bev[:, i * 3 + li] = np.asarray(bb, np.float32)[i]

    nc = _build_kernel(float(dt))
    in_maps = []
    for r in range(N_CORES):
        in_maps.append({
            "x": np.ascontiguousarray(x[2 * r:2 * r + 2, 0]),
            "wc1": wc1, "wc2": wc2, "wc3": wc3, "wc4": wc4,
            "sel": sel, "gv": gv, "bev": bev, "b4v": b4v,
        })
    res = run_bass_kernel_spmd(nc, in_maps, list(range(N_CORES)))
    out = np.zeros((B, 1, H, H), np.float32)
    for r in range(N_CORES):
        out[2 * r:2 * r + 2, 0] = res.results[r]["y"]
    return out


# revision 11
# speedup vs baseline: 7.5201x; 7.5201x over previous
"""DiffNet Trainium2 kernel: 5 iterations of a 4-layer CNN (BN training-mode,
stride-1 maxpools) gating a neighbor-diffusion update on x[16,1,256,256].

Sharding: data-parallel, 2 images per NeuronCore across 8 cores. BN batch
stats are synchronized with one tiny AllGather per BN layer (15 total).

Layout: activations live in SBUF as [128 partitions = 32 channels x 4 groups,
row-major padded slab]. Group g = half*2 + image (h-major so each half is a
contiguous 64-partition range). Convs run as 9 accumulating matmuls (one per
3x3 tap) with shifted rhs access patterns against a block-diagonal stationary.
Pools run on raw conv outputs (valid since the BN affine has positive scale),
fused with the BN normalize+relu into the eviction to the next layer's slab.
"""
import numpy as np
import ml_dtypes

BF16NP = ml_dtypes.bfloat16

# ---------------------------------------------------------------------------
# walrus-workaround patches (this build accepts only ONE sync wait per
# instruction; Tile attaches several). Split extras onto injected NoOps.
# ---------------------------------------------------------------------------
_PATCHED = False


def _install_patches():
    global _PATCHED
    if _PATCHED:
        return
    import concourse.tile as tile
    import concourse.mybir as mybir
    import bass_rust
    from concourse.vector_clock import ScopedClock
    from concourse import tile_utils

    tile_utils.max_sbuf_usage = 207 * 1024  # default 192K leaves room unused

    _orig_lower = tile.TileContext._lower_ordered_insts
    counter = [0]

    def _patched_lower(self, ordered):
        for bb_name, insts in list(ordered.items()):
            new = []
            changed = False
            for inst in insts:
                try:
                    si = inst.sync_info
                except Exception:
                    si = None
                waits = list(si.on_wait) if si is not None else []
                if len(waits) > 1:
                    for w in waits[:-1]:
                        nop = mybir.InstNoOp(name=f"wsplit_{counter[0]}")
                        counter[0] += 1
                        nop.engine = inst.engine
                        nop.sync_info = bass_rust.SyncInfo(on_wait=[w], on_update=[])
                        new.append(nop)
                    inst.sync_info = bass_rust.SyncInfo(
                        on_wait=[waits[-1]], on_update=list(si.on_update)
                    )
                    changed = True
                new.append(inst)
            if changed:
                ordered[bb_name] = new
        return _orig_lower(self, ordered)

    def _patched_drain(self, tick_clock, wait_clock):
        collector = self.nc.sync.drain()
        wait_clock.add_sem_waits(
            collector.ins, ScopedClock({None: tick_clock.global_clock})
        )
        waits = list(collector.ins.sync_info.on_wait)
        if len(waits) > 1:
            collector.ins.sync_info = bass_rust.SyncInfo(
                on_wait=[waits[0]], on_update=[]
            )
            for w in waits[1:]:
                d = self.nc.sync.drain()
                d.ins.sync_info = bass_rust.SyncInfo(on_wait=[w], on_update=[])
        self.nc.all_engine_barrier()
        popped = self.nc._tile_sem_poison_stack.pop()
        assert popped is self._sem_poison
        self.nc.clear_and_free_semaphores(list(self.sems.allocated().values()))
        self.nc.all_engine_barrier()

    tile.TileContext._lower_ordered_insts = _patched_lower
    tile.TileContext._drain_and_barrier = _patched_drain
    _PATCHED = True


# ---------------------------------------------------------------------------
# problem constants
# ---------------------------------------------------------------------------
L = 5
CNN_C = 32
META_C = 5
B = 16
H = 256
EPS = 1e-5
N_CORES = 8
NEG = -1e30

HL = [256, 257, 258, 259]          # conv input/output H (=W) for layers 1..4
AL = [128, 129, 129, 129]          # rows in half 0
BL = [HL[i] - AL[i] for i in range(4)]   # rows in half 1 (128,128,129,130)
RL = [max(AL[i], BL[i]) for i in range(4)]
WPL = [HL[i] + 2 for i in range(4)]
SLAB = (RL[3] + 2) * WPL[3]        # 132*261 = 34452 flat elems


def _build_kernel(dt_val, reps=1):
    import concourse.bass as bass
    import concourse.tile as tile
    from concourse import mybir

    FP32 = mybir.dt.float32
    BF16 = mybir.dt.bfloat16
    ALU = mybir.AluOpType
    ACTF = mybir.ActivationFunctionType
    AXIS = mybir.AxisListType

    nc = bass.Bass("TRN2", target_bir_lowering=False, debug=False,
                   num_devices=N_CORES)

    x_in = nc.dram_tensor("x", [2, 256, 256], FP32, kind="ExternalInput").ap()
    wc1 = nc.dram_tensor("wc1", [4, 45 * 128], BF16, kind="ExternalInput").ap()
    wc2 = nc.dram_tensor("wc2", [128, 45 * 128], BF16, kind="ExternalInput").ap()
    wc3 = nc.dram_tensor("wc3", [128, 45 * 128], BF16, kind="ExternalInput").ap()
    wc4 = nc.dram_tensor("wc4", [128, 45 * 20], BF16, kind="ExternalInput").ap()
    sel_in = nc.dram_tensor("sel", [128, 32], FP32, kind="ExternalInput").ap()
    g_in = nc.dram_tensor("gv", [32, 15], FP32, kind="ExternalInput").ap()
    be_in = nc.dram_tensor("bev", [32, 15], FP32, kind="ExternalInput").ap()
    b4_in = nc.dram_tensor("b4v", [20, 5], FP32, kind="ExternalInput").ap()
    y_out = nc.dram_tensor("y", [2, 256, 256], FP32, kind="ExternalOutput").ap()

    with tile.TileContext(nc) as tc:
        with (
            tc.tile_pool(name="big", bufs=1) as bigp,
            tc.tile_pool(name="wp", bufs=1) as wp,
            tc.tile_pool(name="xp", bufs=2) as xp,
            tc.tile_pool(name="mp", bufs=1) as mp,
            tc.tile_pool(name="sp", bufs=8) as sp,
            tc.tile_pool(name="cv", bufs=3, space="PSUM") as cvp,
            tc.tile_pool(name="upp", bufs=2, space="PSUM") as upp,
            tc.tile_pool(name="slp", bufs=1, space="PSUM") as slp,
            tc.tile_pool(name="dr", bufs=4, space="DRAM") as drp,
        ):
            S1 = bigp.tile([128, SLAB], BF16, tag="S1")
            S2 = bigp.tile([128, SLAB], BF16, tag="S2")
            w1t = wp.tile([4, 45 * 128], BF16)
            w2t = wp.tile([128, 45 * 128], BF16)
            w3t = wp.tile([128, 45 * 128], BF16)
            w4t = wp.tile([128, 45 * 20], BF16)
            selt = wp.tile([128, 32], FP32)
            gt = wp.tile([32, 15], FP32)
            bet = wp.tile([32, 15], FP32)
            b4t = wp.tile([20, 5], FP32)
            epst = wp.tile([32, 1], FP32)
            nc.vector.memset(epst[:], EPS)
            meta = mp.tile([128, 5, 4, 256], BF16)

            nc.sync.dma_start(out=w1t[:], in_=wc1[:])
            nc.sync.dma_start(out=w2t[:], in_=wc2[:])
            nc.sync.dma_start(out=w3t[:], in_=wc3[:])
            nc.sync.dma_start(out=w4t[:], in_=wc4[:])
            nc.sync.dma_start(out=selt[:], in_=sel_in[:])
            nc.sync.dma_start(out=gt[:], in_=g_in[:])
            nc.sync.dma_start(out=bet[:], in_=be_in[:])
            nc.sync.dma_start(out=b4t[:], in_=b4_in[:])

            # ---------------- xgrid setup ----------------
            def refresh_xg_halos(xg):
                xv = xg
                # zero all halo slots first (aligned full-range memsets);
                # the DMAs below overwrite where a real neighbor row exists.
                nc.vector.memset(xv[:, :, 0:1], 0.0)
                nc.vector.memset(xv[:, :, 257:258], 0.0)
                nc.vector.memset(xv[:, 0:1, :], 0.0)
                nc.vector.memset(xv[:, 5:6, :], 0.0)
                for g in range(4):
                    p0 = 32 * g
                    # top halo within group: part q row0 <- part q-1 row4
                    nc.sync.dma_start(out=xv[p0 + 1:p0 + 32, 0:1, :],
                                      in_=xv[p0:p0 + 31, 4:5, :])
                    # bottom halo within group: part q row5 <- part q+1 row1
                    nc.sync.dma_start(out=xv[p0:p0 + 31, 5:6, :],
                                      in_=xv[p0 + 1:p0 + 32, 1:2, :])
                for g in (2, 3):  # h1 first block: top halo <- h0 last row
                    src = (g - 2) * 32 + 31
                    nc.sync.dma_start(out=xv[g * 32:g * 32 + 1, 0:1, :],
                                      in_=xv[src:src + 1, 4:5, :])
                for g in (0, 1):  # h0 last block: bottom halo <- h1 first row
                    dst = g * 32 + 31
                    src = (g + 2) * 32
                    nc.sync.dma_start(out=xv[dst:dst + 1, 5:6, :],
                                      in_=xv[src:src + 1, 1:2, :])

            def build_slab1(xg):
                # write x into S1 partitions 0..3 as the layer-1 input slab
                wp1 = WPL[0]
                s1v = S1[:, 0:130 * wp1].rearrange("p (r c) -> p r c", c=wp1)
                # zero halo rows first (aligned memset), then fill real seams
                nc.vector.memset(s1v[0:4, 0:1, :], 0.0)
                nc.vector.memset(s1v[0:4, 129:130, :], 0.0)
                for g in range(4):
                    nc.gpsimd.dma_start(out=s1v[g:g + 1, 1:129, :],
                                        in_=xg[32 * g:32 * g + 32, 1:5, :])
                nc.gpsimd.dma_start(out=s1v[2:3, 0:1, :], in_=xg[31:32, 4:5, :])
                nc.gpsimd.dma_start(out=s1v[3:4, 0:1, :], in_=xg[63:64, 4:5, :])
                nc.gpsimd.dma_start(out=s1v[0:1, 129:130, :], in_=xg[64:65, 1:2, :])
                nc.gpsimd.dma_start(out=s1v[1:2, 129:130, :], in_=xg[96:97, 1:2, :])

            # ---------------- per-layer emitters ----------------
            def conv_layer(li, it):
                """li: 0..3 (layers 1..4). Reads S1, writes raw into S2."""
                wpad = WPL[li]
                rr = RL[li]
                base = wpad + 1
                total = rr * wpad - 2
                idx0 = it * 9
                if li == 0:
                    wt, kp, mp_ = w1t, 4, 128
                elif li == 1:
                    wt, kp, mp_ = w2t, 128, 128
                elif li == 2:
                    wt, kp, mp_ = w3t, 128, 128
                else:
                    wt, kp, mp_ = w4t, 128, 20
                msz = 128 if li < 3 else 20
                off = 0
                while off < total:
                    n = min(512, total - off)
                    pt = cvp.tile([128, 512], mybir.dt.float32, tag="cv")
                    for t in range(9):
                        dy, dx = t // 3 - 1, t % 3 - 1
                        ro = base + off + dy * wpad + dx
                        nc.tensor.matmul(
                            pt[0:msz, 0:n],
                            wt[0:kp, (idx0 + t) * mp_:(idx0 + t) * mp_ + msz],
                            S1[0:kp, ro:ro + n],
                            start=(t == 0), stop=(t == 8),
                        )
                    if li < 3:
                        nc.scalar.activation(
                            S2[:, base + off:base + off + n], pt[:, 0:n],
                            mybir.ActivationFunctionType.Copy)
                    else:
                        nc.scalar.activation(
                            S2[0:20, base + off:base + off + n], pt[0:20, 0:n],
                            mybir.ActivationFunctionType.Relu,
                            bias=b4t[:, it:it + 1], scale=1.0)
                    off += n

            def stats_pool_norm(li, it):
                """Stats + AllGather + affine; pool raw; normalized -> S1."""
                wpad, rr = WPL[li], RL[li]
                a, b = AL[li], BL[li]
                w = HL[li]
                a2, b2 = AL[li + 1], BL[li + 1]
                r2, wp2 = RL[li + 1], WPL[li + 1]
                w2 = HL[li + 1]
                nfull = 16 * w * w
                s2v = S2[:, 0:(rr + 2) * wpad].rearrange("p (r c) -> p r c", c=wpad)

                # per-partition sums over real pixels (fp32 accumulate)
                sums = sp.tile([128, 2], mybir.dt.float32, tag="sums")
                nc.vector.tensor_reduce(sums[0:64, 0:1], s2v[0:64, 1:1 + a, 1:1 + w],
                                        AXIS.XY, ALU.add)
                nc.vector.tensor_reduce(sums[64:128, 0:1], s2v[64:128, 1:1 + b, 1:1 + w],
                                        AXIS.XY, ALU.add)

                # pool prep on raw: -inf halos
                nc.vector.memset(s2v[:, 1:rr + 1, 0:1], NEG)
                nc.vector.memset(s2v[:, 1:rr + 1, w + 1:w + 2], NEG)
                nc.vector.memset(s2v[0:64, 0:1, :], NEG)          # h0 top pad
                nc.sync.dma_start(out=s2v[64:128, 0:1, :],        # h1 top <- h0 last
                                  in_=s2v[0:64, a:a + 1, :])
                nc.sync.dma_start(out=s2v[0:64, a + 1:a + 2, :],  # h0 bottom <- h1 first
                                  in_=s2v[64:128, 1:2, :])
                if b + 1 <= rr + 1:                               # h1 bottom pad(s)
                    nc.vector.memset(s2v[64:128, b + 1:rr + 2, :], NEG)

                # rowmax into S1 (input slab is dead after the convs)
                s1p = S1[:, 0:(rr + 1) * wpad].rearrange("p (r c) -> p r c", c=wpad)
                nc.vector.tensor_max(s1p[:, 0:rr + 1, :], s2v[:, 0:rr + 1, :],
                                     s2v[:, 1:rr + 2, :])

                # sum of squares: in-place square of raw (raw is dead after rowmax)
                nc.vector.scalar_tensor_tensor(
                    out=s2v[0:64, 1:1 + a, 1:1 + w],
                    in0=s2v[0:64, 1:1 + a, 1:1 + w], scalar=1.0,
                    in1=s2v[0:64, 1:1 + a, 1:1 + w],
                    op0=ALU.mult, op1=ALU.mult, accum_out=sums[0:64, 1:2])
                nc.vector.scalar_tensor_tensor(
                    out=s2v[64:128, 1:1 + b, 1:1 + w],
                    in0=s2v[64:128, 1:1 + b, 1:1 + w], scalar=1.0,
                    in1=s2v[64:128, 1:1 + b, 1:1 + w],
                    op0=ALU.mult, op1=ALU.mult, accum_out=sums[64:128, 1:2])

                # cross-group reduce via selector matmul, then AllGather
                chan = slp.tile([32, 2], mybir.dt.float32, tag="sl")
                nc.tensor.matmul(chan[:, :], selt[:, :], sums[:, :],
                                 start=True, stop=True)
                csb = sp.tile([32, 2], mybir.dt.float32, tag="csb")
                nc.vector.tensor_copy(csb[:], chan[:, :])
                ccin = drp.tile([64], mybir.dt.float32, tag="ccin")
                ccout = drp.tile([512], mybir.dt.float32, tag="ccout")
                nc.gpsimd.dma_start(out=ccin[:], in_=csb[:])
                nc.gpsimd.collective_compute(
                    "AllGather", ALU.bypass,
                    replica_groups=[list(range(N_CORES))],
                    ins=[ccin[:].opt()], outs=[ccout[:].opt()])
                gath = sp.tile([32, 2, 8], mybir.dt.float32, tag="gath")
                oap = ccout[:]
                src = bass.AP(tensor=oap.tensor, offset=oap.offset,
                              ap=[[2, 32], [1, 2], [64, 8]])
                nc.sync.dma_start(out=gath[:], in_=src)
                tot = sp.tile([32, 2], mybir.dt.float32, tag="tot")
                nc.vector.tensor_reduce(tot[:], gath[:], AXIS.X, ALU.add)

                # affine: scale = g / sqrt(var+eps); bias = be - mean*scale
                af = sp.tile([32, 6], mybir.dt.float32, tag="af")
                col = it * 3 + li
                nc.vector.tensor_scalar_mul(af[:, 0:1], tot[:, 0:1], 1.0 / nfull)
                nc.vector.tensor_scalar_mul(af[:, 1:2], tot[:, 1:2], 1.0 / nfull)
                nc.vector.tensor_mul(af[:, 2:3], af[:, 0:1], af[:, 0:1])
                nc.vector.tensor_sub(af[:, 1:2], af[:, 1:2], af[:, 2:3])  # var
                nc.scalar.activation(af[:, 2:3], af[:, 1:2],
                                     ACTF.Sqrt, bias=epst[:], scale=1.0)
                nc.vector.reciprocal(af[:, 3:4], af[:, 2:3])
                aff = sp.tile([32, 2], mybir.dt.float32, tag="aff")
                nc.vector.tensor_mul(aff[:, 0:1], af[:, 3:4], gt[:, col:col + 1])
                nc.vector.tensor_mul(af[:, 4:5], af[:, 0:1], aff[:, 0:1])
                nc.vector.tensor_sub(aff[:, 1:2], bet[:, col:col + 1], af[:, 4:5])
                aff128 = sp.tile([128, 2], mybir.dt.float32, tag="aff128")
                for g in range(4):
                    nc.sync.dma_start(out=aff128[32 * g:32 * g + 32, :], in_=aff[:])

                # colmax: prow (in S1) -> pooled (in S2, next-layer geometry)
                s2n = S2[:, 0:(r2 + 2) * wp2].rearrange("p (r c) -> p r c", c=wp2)
                nc.vector.tensor_max(s2n[0:64, 1:1 + a2, 1:1 + w2],
                                     s1p[0:64, 0:a2, 0:w2],
                                     s1p[0:64, 0:a2, 1:1 + w2])
                d = a2 - a
                nc.vector.tensor_max(s2n[64:128, 1:1 + b2, 1:1 + w2],
                                     s1p[64:128, d:d + b2, 0:w2],
                                     s1p[64:128, d:d + b2, 1:1 + w2])

                # normalize+relu pooled -> S1 (next input slab)
                s1n = S1[:, 0:(r2 + 2) * wp2].rearrange("p (r c) -> p r c", c=wp2)
                nc.scalar.activation(s1n[0:64, 1:1 + a2, 1:1 + w2],
                                     s2n[0:64, 1:1 + a2, 1:1 + w2],
                                     ACTF.Relu, bias=aff128[0:64, 1:2],
                                     scale=aff128[0:64, 0:1])
                nc.scalar.activation(s1n[64:128, 1:1 + b2, 1:1 + w2],
                                     s2n[64:128, 1:1 + b2, 1:1 + w2],
                                     ACTF.Relu, bias=aff128[64:128, 1:2],
                                     scale=aff128[64:128, 0:1])

                # next-slab halos (zeros for conv pad; cross copies at half seam)
                nc.vector.memset(s1n[:, 1:r2 + 1, 0:1], 0.0)
                nc.vector.memset(s1n[:, 1:r2 + 1, w2 + 1:w2 + 2], 0.0)
                nc.vector.memset(s1n[0:64, 0:1, :], 0.0)
                if b2 + 1 <= r2 + 1:
                    nc.vector.memset(s1n[64:128, b2 + 1:r2 + 2, :], 0.0)
                nc.sync.dma_start(out=s1n[64:128, 0:1, :],
                                  in_=s1n[0:64, a2:a2 + 1, :])
                nc.sync.dma_start(out=s1n[0:64, a2 + 1:a2 + 2, :],
                                  in_=s1n[64:128, 1:2, :])

            def repack_meta(it):
                wp4 = WPL[3]
                for g in range(4):
                    hh = g >> 1
                    row0 = (2 if hh == 0 else 1)
                    for k in range(5):
                        p = 5 * g + k
                        src = S2[p:p + 1, row0 * wp4:(row0 + 128) * wp4] \
                            .rearrange("p (j rr c) -> p j rr c", rr=4, c=wp4)
                        nc.sync.dma_start(
                            out=meta[32 * g:32 * g + 32, k, :, :],
                            in_=src[:, :, :, 2:258])

            def update_x(it, xg_cur):
                xn = xp.tile([128, 6, 258], mybir.dt.float32, tag="xg")
                xc = xg_cur[:, 1:5, 1:257]
                up = xg_cur[:, 0:4, 1:257]
                dn = xg_cur[:, 2:6, 1:257]
                lf = xg_cur[:, 1:5, 2:258]
                rt = xg_cur[:, 1:5, 0:256]
                acc = upp.tile([128, 4, 256], mybir.dt.float32, tag="up")
                tmp = xp.tile([128, 4, 256], mybir.dt.float32, tag="ut")
                STT = nc.vector.scalar_tensor_tensor
                STT(out=acc[:], in0=meta[:, 1, :, :], scalar=dt_val, in1=up,
                    op0=ALU.mult, op1=ALU.mult)
                STT(out=tmp[:], in0=meta[:, 2, :, :], scalar=dt_val, in1=dn,
                    op0=ALU.mult, op1=ALU.mult)
                nc.vector.tensor_add(acc[:], acc[:], tmp[:])
                STT(out=tmp[:], in0=meta[:, 3, :, :], scalar=dt_val, in1=lf,
                    op0=ALU.mult, op1=ALU.mult)
                nc.vector.tensor_add(acc[:], acc[:], tmp[:])
                STT(out=tmp[:], in0=meta[:, 4, :, :], scalar=dt_val, in1=rt,
                    op0=ALU.mult, op1=ALU.mult)
                nc.vector.tensor_add(acc[:], acc[:], tmp[:])
                STT(out=tmp[:], in0=meta[:, 0, :, :], scalar=dt_val, in1=xc,
                    op0=ALU.mult, op1=ALU.mult)
                nc.vector.tensor_add(tmp[:], tmp[:], xc)
                nc.vector.tensor_sub(xn[:, 1:5, 1:257], tmp[:], acc[:])
                if it == L - 1:
                    nc.vector.tensor_scalar_max(xn[:, 1:5, 1:257],
                                                xn[:, 1:5, 1:257], 0.0)
                return xn

            # ---------------- main loop ----------------
            for rep in range(reps):
                xg = xp.tile([128, 6, 258], FP32, tag="xg")
                nc.vector.memset(xg[:], 0.0)
                for g in range(4):
                    hh, img = g >> 1, g & 1
                    nc.sync.dma_start(
                        out=xg[32 * g:32 * g + 32, 1:5, 1:257],
                        in_=x_in[img, hh * 128:(hh + 1) * 128, :])
                refresh_xg_halos(xg)
                build_slab1(xg)
                for it in range(L):
                    for li in range(3):
                        conv_layer(li, it)
                        stats_pool_norm(li, it)
                    conv_layer(3, it)
                    repack_meta(it)
                    xg_new = update_x(it, xg)
                    if it == L - 1:
                        for g in range(4):
                            hh, img = g >> 1, g & 1
                            nc.sync.dma_start(
                                out=y_out[img, hh * 128:(hh + 1) * 128, :],
                                in_=xg_new[32 * g:32 * g + 32, 1:5, 1:257])
                    else:
                        refresh_xg_halos(xg_new)
                        build_slab1(xg_new)
                    xg = xg_new

    return nc


def _prep_inputs(w1, w2, w3, w4, b4):
    """Build device-side stationary layouts (bf16) from torch-layout weights."""
    wc1 = np.zeros((4, 45, 128), np.float32)
    wc2 = np.zeros((128, 45, 128), np.float32)
    wc3 = np.zeros((128, 45, 128), np.float32)
    wc4 = np.zeros((128, 45, 20), np.float32)
    for i in range(L):
        for t in range(9):
            dy, dx = t // 3, t % 3
            s = i * 9 + t
            for g in range(4):
                wc1[g, s, 32 * g:32 * g + 32] = w1[i, :, 0, dy, dx]
                wc2[32 * g:32 * g + 32, s, 32 * g:32 * g + 32] = \
                    w2[i, :, :, dy, dx].T
                wc3[32 * g:32 * g + 32, s, 32 * g:32 * g + 32] = \
                    w3[i, :, :, dy, dx].T
                wc4[32 * g:32 * g + 32, s, 5 * g:5 * g + 5] = \
                    w4[i, :, :, dy, dx].T
    sel = np.zeros((128, 32), np.float32)
    for p in range(128):
        sel[p, p % 32] = 1.0
    b4v = np.zeros((20, 5), np.float32)
    for i in range(L):
        for g in range(4):
            b4v[5 * g:5 * g + 5, i] = b4[i]
    return (wc1.reshape(4, -1).astype(BF16NP),
            wc2.reshape(128, -1).astype(BF16NP),
            wc3.reshape(128, -1).astype(BF16NP),
            wc4.reshape(128, -1).astype(BF16NP),
            sel, b4v)


_NC_CACHE = {}


def _run(x, w1, g1, be1, w2, g2, be2, w3, g3, be3, w4, b4, dt, reps=1):
    _install_patches()
    from concourse.bass_utils import run_bass_kernel_spmd

    x = np.asarray(x, np.float32)
    wc1, wc2, wc3, wc4, sel, b4v = _prep_inputs(
        np.asarray(w1, np.float32), np.asarray(w2, np.float32),
        np.asarray(w3, np.float32), np.asarray(w4, np.float32),
        np.asarray(b4, np.float32))
    # conv biases b1..b3 cancel inside train-mode BN; g/be enter the affine.
    gv = np.zeros((32, 15), np.float32)
    bev = np.zeros((32, 15), np.float32)
    for i in range(L):
        for li, (gg, bb) in enumerate(((g1, be1), (g2, be2), (g3, be3))):
            gv[:, i * 3 + li] = np.asarray(gg, np.float32)[i]
            bev[:, i * 3 + li] = np.asarray(bb, np.float32)[i]

    key = (float(dt), reps)
    if key not in _NC_CACHE:
        _NC_CACHE[key] = _build_kernel(float(dt), reps)
    nc = _NC_CACHE[key]
    in_maps = []
    for r in range(N_CORES):
        in_maps.append({
            "x": np.ascontiguousarray(x[2 * r:2 * r + 2, 0]),
            "wc1": wc1, "wc2": wc2, "wc3": wc3, "wc4": wc4,
            "sel": sel, "gv": gv, "bev": bev, "b4v": b4v,
        })
    res = run_bass_kernel_spmd(nc, in_maps, list(range(N_CORES)))
    out = np.zeros((B, 1, H, H), np.float32)
    for r in range(N_CORES):
        out[2 * r:2 * r + 2, 0] = res.results[r]["y"]
    return out


def kernel(x, w1, b1, g1, be1, w2, b2, g2, be2, w3, b3, g3, be3, w4, b4, dt):
    return _run(x, w1, g1, be1, w2, g2, be2, w3, g3, be3, w4, b4, dt, reps=1)
